# revision 11
# baseline (speedup 1.0000x reference)
"""Trainium2 distributed kernel for nn_BDEN_16621523435784 (binarized dense CNN).

Strategy (data parallel, 1 sample per NeuronCore, 8 cores):
- Layer 0 (3-channel f32 input conv) is computed on host with the exact same
  eager-jax ops as the reference (bit-exact; ~0.1% of total FLOPs). The network
  is chaotic: one flipped sign anywhere changes the final output completely, so
  everything must be bit-exact.
- Layers 1..8 on device: activations/weights are +-1 (fp8, exact), conv sums are
  exact integers (fp32 PSUM accumulate). BN+sign collapses to an integer
  threshold compare per channel, fused into one ScalarE Sign activation
  (bias = +-(0.5 - T_c), scale = +-1), thresholds precomputed on host with
  bit-exact jax arithmetic.
- Layer 9 produces f32 logits (affine on integer sums), written into per-band
  AllToAll buffers (output row y belongs to core y mod 8), 8 pipelined
  AllToAlls + on-device softmax over the batch axis.
"""

import os

os.environ.setdefault("JAX_PLATFORMS", "")

import numpy as np
import ml_dtypes

import concourse.bass as bass
import concourse.tile as tile
from concourse import bacc, mybir
from concourse.bass_utils import run_bass_kernel_spmd

FP8 = mybir.dt.float8e4
F32 = mybir.dt.float32
NP8 = ml_dtypes.float8_e4m3
AF = mybir.ActivationFunctionType
DR = mybir.MatmulPerfMode.DoubleRow

NCORES = 8
EPS = 1e-5

# (cin, cout, stride, transposed) for the 10 binarized conv layers
CFG = [(3, 64, 2, False), (64, 64, 1, False), (64, 128, 2, False), (128, 128, 1, False),
       (128, 256, 1, False), (256, 256, 2, True), (256, 128, 1, False), (128, 128, 2, True),
       (128, 64, 1, False), (64, 19, 1, False)]

# tconv output-parity classes: (py, px, [taps (a, b, da, db)], doublerow?)
# out(2u+py, 2v+px) = sum_taps wf[:, :, a, b] @ in[u+da, v+db]
TCONV_CLASSES = [
    (0, 0, [[(1, 1, 0, 0)]]),
    (0, 1, [[(1, 0, 0, 0)], [(1, 2, 0, 1)]]),
    (1, 0, [[(0, 1, 0, 0), (2, 1, 1, 0)]]),
    (1, 1, [[(0, 0, 0, 0), (2, 0, 1, 0)], [(0, 2, 0, 1), (2, 2, 1, 1)]]),
]


def pad16(x):
    return (x + 15) // 16 * 16


def make_geom(H0):
    g = {}
    H = H0 // 2
    sizes = [H]
    for l in range(1, 10):
        cin, cout, s, tr = CFG[l]
        H = H * 2 if tr else H // s
        sizes.append(H)
    for l in range(1, 10):
        cin, cout, s, tr = CFG[l]
        Hin, Hout = sizes[l - 1], sizes[l]
        d = dict(cin=cin, cout=cout, s=s, tr=tr, Hin=Hin, Win=Hin, Hout=Hout,
                 Wout=Hout, Wpin=pad16(Hin + 2), Wpout=pad16(Hout + 2),
                 dup=(cin == 64))
        if tr:
            d['Wq'] = Hout // 2
            d['nq'] = max(1, min(512 // d['Wq'], Hout // 2))
        else:
            d['nr'] = max(1, min(512 // Hout, Hout))
        if l == 5:
            B = Hout
        elif l in (1, 7):
            B = min(128, Hout)
        elif l in (2, 6, 8, 9):
            B = min(64, Hout)
        else:
            B = Hout
        d['B'] = B
        if tr:
            d['SRq'] = min(8, B // 2)
            d['nq'] = min(d['nq'], d['SRq'])
        elif l == 8:
            d['SRp'] = min(8, B // 2)
        elif l != 9:
            d['SR'] = min(max(d['nr'] * 4, 8), B)
        g[l] = d
    g['sizes'] = sizes
    g['H0'] = H0
    return g


# ---------------------------------------------------------------------------
# Stream plans. A stream: (wkey_suffix si, kind, K, row_off, col_off, jrows)
#   kind 'dr'/'dupdr' use DoubleRow; row_off = tile-row offset (ky/da term),
#   col_off = buffer-col offset (kx/db term), jrows = j-dim step in input rows.
# ---------------------------------------------------------------------------

def conv_streams(l):
    """L3/L4/L6 per-slab streams: 3 DoubleRow (ky 0+1) + 3 plain (ky=2)."""
    out = []
    for kx in range(3):
        out.append(dict(kind='dr', K=128, p0=0, row_off=0, col_off=kx, jrows=1))
    for kx in range(3):
        out.append(dict(kind='plain', K=128, p0=0, row_off=2, col_off=kx, jrows=None,
                        ky=2, kx=kx))
    return out


def dupdr_streams():
    return [dict(kind='dupdr', K=128, p0=0, row_off=0, col_off=kx, jrows=2)
            for kx in range(3)]


def l8_streams():
    return [dict(kind='plain', K=128, p0=0, row_off=ky, col_off=kx, jrows=None,
                 ky=ky, kx=kx) for ky in range(3) for kx in range(3)]


def l9_streams():
    out = []
    for kx in range(3):
        out.append(dict(kind='dupk128', K=128, p0=0, row_off=0, col_off=kx, jrows=None))
    for kx in range(3):
        out.append(dict(kind='dupk64', K=64, p0=0, row_off=2, col_off=kx, jrows=None))
    return out


# ---------------------------------------------------------------------------
# Host-side preparation
# ---------------------------------------------------------------------------

def _host_l0_and_thresholds(inputs):
    import jax
    import jax.numpy as jnp
    from jax import lax

    cpu = jax.devices("cpu")[0]
    with jax.default_device(cpu):
        return _host_l0_impl(inputs, jnp, lax)


def _host_l0_impl(inputs, jnp, lax):

    DN = ('NCHW', 'OIHW', 'NCHW')
    x = jnp.asarray(inputs['x'])
    w0 = jnp.asarray(inputs['w0'])
    bnp0 = jnp.asarray(inputs['bnp0'])

    wb0 = jnp.sign(w0)
    h = lax.conv_general_dilated(x, wb0, (2, 2), ((1, 1), (1, 1)),
                                 dimension_numbers=DN)
    g, b, m, v = bnp0[0], bnp0[1], bnp0[2], bnp0[3]
    h = g[:, None, None] * (h - m[:, None, None]) * lax.rsqrt(v[:, None, None] + EPS) \
        + b[:, None, None]
    a0 = np.asarray(jnp.sign(h), dtype=np.float32)

    th = {}
    for l in range(1, 9):
        cin, cout, s, tr = CFG[l]
        Kmax = cin * 9
        bnp = jnp.asarray(inputs[f'bnp{l}'])
        ks = jnp.arange(-Kmax, Kmax + 1, dtype=jnp.float32)
        xg = jnp.broadcast_to(ks[None, :, None], (cout, 2 * Kmax + 1, 1))
        g, b, m, v = bnp[0], bnp[1], bnp[2], bnp[3]
        vals = g[:, None, None] * (xg - m[:, None, None]) \
            * lax.rsqrt(v[:, None, None] + EPS) + b[:, None, None]
        sg = np.asarray(jnp.sign(vals))[:, :, 0]
        assert not (sg == 0).any(), f"layer {l}: bn output exactly 0 on integer grid"
        pos = sg > 0
        scale = np.empty(cout, np.float32)
        bias = np.empty(cout, np.float32)
        for c in range(cout):
            p = pos[c]
            if p.all():
                scale[c], bias[c] = 1.0, float(Kmax + 1.5)
            elif not p.any():
                scale[c], bias[c] = 1.0, float(-(Kmax + 1.5))
            else:
                dd = np.diff(p.astype(np.int8))
                nz = np.nonzero(dd)[0]
                assert len(nz) == 1, f"layer {l} ch {c}: non-monotone bn sign"
                idx = int(nz[0])
                if dd[idx] == 1:   # sign=+1 iff k >= T
                    T = float(idx + 1 - Kmax)
                    scale[c], bias[c] = 1.0, 0.5 - T
                else:              # sign=+1 iff k <= T
                    T = float(idx - Kmax)
                    scale[c], bias[c] = -1.0, T + 0.5
        th[l] = (scale, bias)

    bnp9 = np.asarray(inputs['bnp9'], np.float64)
    g, b, m, v = bnp9[0], bnp9[1], bnp9[2], bnp9[3]
    s9 = (g / np.sqrt(v + EPS)).astype(np.float32)
    t9 = (b - m * g / np.sqrt(v + EPS)).astype(np.float32)
    return a0, th, s9, t9


def _weight_arrays(inputs):
    """All stream lhsT arrays (fp8) flat-packed; offsets[(l,half,slab,ci,si)]."""
    blocks, offsets, cursor = [], {}, [0]

    def emit(key, arr):
        a = np.ascontiguousarray(arr.astype(NP8))
        offsets[key] = (cursor[0], a.shape)
        blocks.append(a.reshape(-1))
        cursor[0] += a.size

    for l in range(1, 10):
        cin, cout, s, tr = CFG[l]
        w = np.sign(np.asarray(inputs[f'w{l}'], np.float32))
        if tr:
            wf = np.flip(w, (2, 3)).transpose(1, 0, 2, 3)  # [cout, cin, 3, 3]
            for half in range(cout // 128):
                M = 128
                hs = slice(128 * half, 128 * half + M)
                for slab in range(cin // 128):
                    ss = slice(128 * slab, 128 * slab + 128)
                    for ci, (py, px, streams) in enumerate(TCONV_CLASSES):
                        for si, taps in enumerate(streams):
                            if len(taps) == 1:
                                a, b_, da, db = taps[0]
                                emit((l, half, slab, ci, si), wf[hs, ss, a, b_].T)
                            else:
                                arr = np.zeros((128, 2, M), np.float32)
                                for j, (a, b_, da, db) in enumerate(taps):
                                    arr[:, j, :] = wf[hs, ss, a, b_].T
                                emit((l, half, slab, ci, si), arr)
        elif l in (1, 2):
            for half in range((cout + 127) // 128):
                M = min(128, cout - 128 * half)
                hs = slice(128 * half, 128 * half + M)
                for si, st in enumerate(dupdr_streams()):
                    kx = st['col_off']
                    arr = np.zeros((128, 2, M), np.float32)
                    arr[:64, 0, :] = w[hs, :, 0, kx].T
                    arr[:64, 1, :] = w[hs, :, 2, kx].T
                    arr[64:, 0, :] = w[hs, :, 1, kx].T
                    emit((l, half, 0, 0, si), arr)
        elif l == 8:
            for si, st in enumerate(l8_streams()):
                emit((8, 0, 0, 0, si), w[:, :, st['ky'], st['kx']].T)
        elif l == 9:
            for si, st in enumerate(l9_streams()):
                kx = st['col_off']
                if st['kind'] == 'dupk128':
                    arr = np.zeros((128, 19), np.float32)
                    arr[:64] = w[:, :, 0, kx].T
                    arr[64:] = w[:, :, 1, kx].T
                else:
                    arr = w[:, :, 2, kx].T
                emit((9, 0, 0, 0, si), arr)
        else:  # L3, L4, L6
            for half in range((cout + 127) // 128):
                M = min(128, cout - 128 * half)
                hs = slice(128 * half, 128 * half + M)
                for slab in range(max(1, cin // 128)):
                    ss = slice(128 * slab, 128 * slab + 128)
                    for si, st in enumerate(conv_streams(l)):
                        if st['kind'] == 'dr':
                            kx = st['col_off']
                            arr = np.zeros((128, 2, M), np.float32)
                            arr[:, 0, :] = w[hs, ss, 0, kx].T
                            arr[:, 1, :] = w[hs, ss, 1, kx].T
                        else:
                            arr = w[hs, ss, st['ky'], st['kx']].T
                        emit((l, half, slab, 0, si), arr)
    wpack = np.concatenate(blocks)
    if wpack.size % 16:
        wpack = np.concatenate([wpack, np.zeros(16 - wpack.size % 16, NP8)])
    return wpack, offsets


def _bias_pack(th, s9, t9):
    cols, colidx = [], {}

    def add(name, vec):
        colidx[name] = len(cols)
        v = np.zeros(128, np.float32)
        v[:len(vec)] = vec
        cols.append(v)

    for l in range(1, 9):
        scale, bias = th[l]
        cout = CFG[l][1]
        if l == 8:
            # col-tiling x2: row pair shares the bank, channels replicated at +64
            add(('scale', 8, 0), np.concatenate([scale, scale]))
            add(('bias', 8, 0), np.concatenate([bias, bias]))
            continue
        for half in range((cout + 127) // 128):
            sl = slice(128 * half, min(cout, 128 * half + 128))
            add(('scale', l, half), scale[sl])
            add(('bias', l, half), bias[sl])
    s9r, t9r = np.zeros(128, np.float32), np.zeros(128, np.float32)
    for c in range(4):
        s9r[32 * c:32 * c + 19] = s9
        t9r[32 * c:32 * c + 19] = t9
    add(('scale', 9, 0), s9r)
    add(('bias', 9, 0), t9r)
    return np.stack(cols, axis=1), colidx


def _pack_a0(a0):
    H = a0.shape[2]
    Wp = pad16(H + 2)
    out = np.zeros((8, 64, H + 4, Wp), NP8)
    out[:, :, 1:H + 1, 1:H + 1] = a0.astype(NP8)
    return out


def with_jdim(ap, step_elems):
    """Insert dim 1 with [step_elems, 2] into a 3D ap [K, rows, cols]."""
    shp = ap.shape
    a = ap.unsqueeze(1).broadcast_to((shp[0], 2) + tuple(shp[1:]))
    a.ap[1] = [step_elems, 2]
    return a


# ---------------------------------------------------------------------------
# Device graph
# ---------------------------------------------------------------------------

def build_nc(geom, woffsets, colidx, nbias, nwpack, dbg=False):
    Hlog = geom['sizes'][9]
    nc = bacc.Bacc("TRN2", target_bir_lowering=False, debug=False)

    a0_p = nc.declare_dram_parameter(
        "a0", [64, geom[1]['Hin'] + 4, geom[1]['Wpin']], FP8, isOutput=False)
    wpack_p = nc.declare_dram_parameter("wpack", [nwpack], FP8, isOutput=False)
    bpack_p = nc.declare_dram_parameter("bpack", [128, nbias], F32, isOutput=False)
    nb9_ = geom[9]['Hout'] // geom[9]['B']
    sub_ = geom[9]['B'] // 8
    out_p = nc.declare_dram_parameter("out", [nb9_, 8, 19, sub_, Hlog], F32,
                                      isOutput=True)

    bufs = {0: a0_p}
    for l in range(1, 9):
        d = geom[l]
        bufs[l] = nc.dram_tensor(f"act{l}", [d['cout'], d['Hout'] + 4, d['Wpout']], FP8)
    nb9 = geom[9]['Hout'] // geom[9]['B']
    sub = geom[9]['B'] // 8
    a2a_in = [nc.dram_tensor(f"a2ain{b}", [8, 19, sub, Hlog], F32) for b in range(nb9)]
    a2a_out = [nc.dram_tensor(f"a2aout{b}", [8, 19, sub, Hlog], F32) for b in range(nb9)]
    dbg_p = {}
    if dbg:
        for l in range(1, 9):
            d = geom[l]
            dbg_p[l] = nc.declare_dram_parameter(
                f"dbg{l}", [d['cout'], d['Hout'] + 4, d['Wpout']], FP8, isOutput=True)
        dbg_p[9] = nc.declare_dram_parameter(
            "dbg9", [nb9, 8, 19, sub, Hlog], F32, isOutput=True)

    with tile.TileContext(nc) as tc:
        with (
            tc.tile_pool(name="weights", bufs=1) as wpool,
            tc.tile_pool(name="psum", bufs=6, space=bass.MemorySpace.PSUM) as psum,
        ):
            # --- weights + bias ---
            wt = {}
            for key, (off, shp) in woffsets.items():
                t = wpool.tile(list(shp), FP8, tag=f"w{key}")
                flat = int(np.prod(shp))
                src = wpack_p.ap()[off:off + flat].rearrange("(k r) -> k r", k=shp[0])
                dst = t[:] if len(shp) == 2 else t[:].rearrange("k j m -> k (j m)")
                nc.sync.dma_start(dst, src)
                wt[key] = t
            bp = wpool.tile([128, nbias], F32, tag="bpack")
            nc.sync.dma_start(bp[:], bpack_p[:])

            # --- zero halo rows of act buffers ---
            zt = wpool.tile([128, 3 * 544], FP8, tag="zeros")
            nc.gpsimd.memset(zt[:], 0.0)
            for l in range(1, 9):
                d = geom[l]
                for slab in range(0, d['cout'], 128):
                    C = min(128, d['cout'] - slab)
                    bb = bufs[l].ap()[slab:slab + C]
                    Wp, H = d['Wpout'], d['Hout']
                    nc.sync.dma_start(
                        bb[:, 0:1, :].rearrange("c a b -> c (a b)"), zt[0:C, 0:Wp])
                    nc.sync.dma_start(
                        bb[:, H + 1:H + 4, :].rearrange("c a b -> c (a b)"),
                        zt[0:C, 0:3 * Wp])

            def rhs_ap(X, st, d, local_row, nr, W, stride):
                r0 = local_row + st['row_off']
                c0 = st['col_off']
                base = X[st['p0']:st['p0'] + st['K'],
                         r0: r0 + stride * (nr - 1) + 1: stride,
                         c0: c0 + stride * (W - 1) + 1: stride]
                if st['jrows'] is not None:
                    return with_jdim(base, st['jrows'] * d['Wpin'])
                return base

            def act_sign(l, half, ps_ap, out_ap, M):
                ic = colidx[('scale', l, half)]
                jc = colidx[('bias', l, half)]
                nc.scalar.activation(out_ap, ps_ap, AF.Sign,
                                     bias=bp[0:M, jc:jc + 1], scale=bp[0:M, ic:ic + 1])

            # ----------------------------------------------------------------
            # layers 1..8
            for l in range(1, 9):
                d = geom[l]
                cin, cout, s, tr = CFG[l]
                with (
                    tc.tile_pool(name=f"in{l}", bufs=2) as inp,
                    tc.tile_pool(name=f"stg{l}", bufs=3) as stg,
                ):
                    if tr:
                        _emit_tconv(nc, tc, geom, l, bufs, wt, psum, inp, stg,
                                    rhs_ap, act_sign)
                    elif l == 8:
                        _emit_l8(nc, geom, bufs, wt, psum, inp, stg, rhs_ap, act_sign)
                    else:
                        _emit_conv(nc, geom, l, bufs, wt, psum, inp, stg,
                                   rhs_ap, act_sign)

            # ----------------------------------------------------------------
            # layer 9 + pipelined A2A + softmax
            d = geom[9]
            with (
                tc.tile_pool(name="in9", bufs=2) as inp,
                tc.tile_pool(name="stg9", bufs=4) as stg,
                tc.tile_pool(name="smx", bufs=2) as smx,
            ):
                B9 = d['B']
                ic9 = colidx[('scale', 9, 0)]
                jc9 = colidx[('bias', 9, 0)]
                for bi, y0 in enumerate(range(0, d['Hout'], B9)):
                    Rt = B9 + 2
                    X = inp.tile([128, Rt, d['Wpin']], FP8, tag="in9")
                    nc.sync.dma_start(X[0:64, :, :], bufs[8].ap()[:, y0:y0 + Rt, :])
                    nc.sync.dma_start(X[64:128, :, :],
                                      bufs[8].ap()[:, y0 + 1:y0 + 1 + Rt, :])
                    W9 = d['Wout']
                    for cy in range(y0, y0 + B9, 4):
                        ps = psum.tile([128, 512], F32, tag="ps")
                        streams = l9_streams()
                        for c in range(4):
                            for si, st in enumerate(streams):
                                rhs = rhs_ap(X, st, d, cy + c - y0, 1, W9, 1)
                                nc.tensor.matmul(
                                    ps[32 * c:32 * c + 19, 0:W9],
                                    wt[(9, 0, 0, 0, si)][:], rhs,
                                    start=(si == 0), stop=(si == len(streams) - 1),
                                    tile_position=(0, 32 * c),
                                    skip_group_check=True)
                        st_t = stg.tile([128, 512], F32, tag="stg9")
                        nc.scalar.activation(st_t[:, 0:W9], ps[:, 0:W9], AF.Identity,
                                             bias=bp[:, jc9:jc9 + 1],
                                             scale=bp[:, ic9:ic9 + 1])
                        for c in range(4):
                            y = cy + c
                            j = y % 8
                            r = (y - y0 - j) // 8
                            dst = a2a_in[bi].ap()[j:j + 1, :, r:r + 1, :].squeeze()
                            nc.sync.dma_start(dst, st_t[32 * c:32 * c + 19, 0:W9])
                    nc.gpsimd.collective_compute(
                        "AllToAll", mybir.AluOpType.bypass,
                        replica_groups=[list(range(NCORES))],
                        ins=[a2a_in[bi].ap().opt()],
                        outs=[a2a_out[bi].ap().opt()],
                    )
                    # softmax over samples for this band
                    F = 19 * sub * Hlog
                    assert F % 128 == 0
                    FD = F // 128
                    Ts = []
                    for sx in range(8):
                        t = smx.tile([128, FD], F32, tag=f"sm_in{sx}")
                        nc.sync.dma_start(
                            t[:], a2a_out[bi].ap()[sx:sx + 1].rearrange(
                                "s a b c -> (s a b c)").rearrange("(p f) -> p f", p=128))
                        Ts.append(t)
                    mx = smx.tile([128, FD], F32, tag="sm_mx")
                    nc.vector.tensor_max(mx[:], Ts[0][:], Ts[1][:])
                    for sx in range(2, 8):
                        nc.vector.tensor_max(mx[:], mx[:], Ts[sx][:])
                    Es = []
                    for sx in range(8):
                        e = smx.tile([128, FD], F32, tag=f"sm_e{sx}")
                        nc.vector.tensor_sub(e[:], Ts[sx][:], mx[:])
                        nc.scalar.activation(e[:], e[:], AF.Exp)
                        Es.append(e)
                    S = smx.tile([128, FD], F32, tag="sm_s")
                    nc.vector.tensor_add(S[:], Es[0][:], Es[1][:])
                    for sx in range(2, 8):
                        nc.vector.tensor_add(S[:], S[:], Es[sx][:])
                    R = smx.tile([128, FD], F32, tag="sm_r")
                    nc.vector.reciprocal(R[:], S[:])
                    for sx in range(8):
                        nc.vector.tensor_mul(Es[sx][:], Es[sx][:], R[:])
                        nc.sync.dma_start(
                            out_p.ap()[bi:bi + 1, sx:sx + 1].rearrange(
                                "x s a b c -> (x s a b c)").rearrange(
                                "(p f) -> p f", p=128),
                            Es[sx][:])

            if dbg:
                for l in range(1, 9):
                    nc.sync.dma_start(dbg_p[l].ap(), bufs[l].ap())
                for b in range(nb9):
                    nc.sync.dma_start(dbg_p[9].ap()[b:b + 1].squeeze(), a2a_in[b].ap())

    nc.compile()
    return nc


def _emit_conv(nc, geom, l, bufs, wt, psum, inp, stg, rhs_ap, act_sign):
    """L1, L2 (dup+DoubleRow), L3, L4, L6 (DoubleRow pairs + plain)."""
    d = geom[l]
    cin, cout, s, tr = CFG[l]
    nhalf = (cout + 127) // 128
    nslab = max(1, cin // 128)
    nr, SR = d['nr'], d['SR']
    for y0 in range(0, d['Hout'], d['B']):
        br0 = s * y0
        Rt = s * (d['B'] - 1) + 3
        Xs = []
        if d['dup']:
            X = inp.tile([128, Rt, d['Wpin']], FP8, tag=f"in{l}")
            nc.sync.dma_start(X[0:64, :, :], bufs[l - 1].ap()[:, br0:br0 + Rt, :])
            nc.sync.dma_start(X[64:128, :, :],
                              bufs[l - 1].ap()[:, br0 + 1:br0 + 1 + Rt, :])
            Xs = [X]
        else:
            for slab in range(nslab):
                X = inp.tile([128, Rt, d['Wpin']], FP8, tag=f"in{l}_{slab}")
                nc.sync.dma_start(
                    X[:], bufs[l - 1].ap()[128 * slab:128 * slab + 128,
                                           br0:br0 + Rt, :])
                Xs.append(X)
        for st0 in range(y0, y0 + d['B'], SR):
            stgs = []
            for half in range(nhalf):
                M = min(128, cout - 128 * half)
                t = stg.tile([128, SR, d['Wpout']], FP8, tag=f"stg{l}_{half}")
                nc.gpsimd.memset(t[0:M, :, 0:1], 0.0)
                nc.gpsimd.memset(t[0:M, :, d['Wout'] + 1:], 0.0)
                stgs.append(t)
            for cy in range(st0, st0 + SR, nr):
                for half in range(nhalf):
                    M = min(128, cout - 128 * half)
                    ps = psum.tile([128, 512], F32, tag="ps")
                    work = []
                    for slab in range(nslab):
                        sts = dupdr_streams() if d['dup'] else conv_streams(l)
                        for si, st in enumerate(sts):
                            work.append((slab, si, st))
                    for i, (slab, si, st) in enumerate(work):
                        rhs = rhs_ap(Xs[slab], st, d, s * (cy - y0), nr, d['Wout'], s)
                        nc.tensor.matmul(
                            ps[0:M, 0:nr * d['Wout']],
                            wt[(l, half, slab, 0, si)][:], rhs,
                            start=(i == 0), stop=(i == len(work) - 1),
                            perf_mode=(DR if st['jrows'] is not None else None))
                    act_sign(l, half,
                             ps[0:M, 0:nr * d['Wout']].rearrange(
                                 "m (r w) -> m r w", r=nr),
                             stgs[half][0:M, cy - st0:cy - st0 + nr, 1:1 + d['Wout']],
                             M)
            for half in range(nhalf):
                M = min(128, cout - 128 * half)
                nc.sync.dma_start(
                    bufs[l].ap()[128 * half:128 * half + M, 1 + st0:1 + st0 + SR, :],
                    stgs[half][0:M, :, :])


def _emit_l8(nc, geom, bufs, wt, psum, inp, stg, rhs_ap, act_sign):
    """L8: 128->64, col-tiling x2 (two output rows per psum bank)."""
    d = geom[8]
    SRp = d['SRp']
    streams = l8_streams()
    for y0 in range(0, d['Hout'], d['B']):
        Rt = d['B'] + 2
        X = inp.tile([128, Rt, d['Wpin']], FP8, tag="in8")
        nc.sync.dma_start(X[:], bufs[7].ap()[:, y0:y0 + Rt, :])
        for st0 in range(y0, y0 + d['B'], 2 * SRp):
            t = stg.tile([128, SRp, d['Wpout']], FP8, tag="stg8")
            nc.gpsimd.memset(t[:, :, 0:1], 0.0)
            nc.gpsimd.memset(t[:, :, d['Wout'] + 1:], 0.0)
            for cy in range(st0, st0 + 2 * SRp, 2):
                ps = psum.tile([128, 512], F32, tag="ps")
                W8 = d['Wout']
                for si, st in enumerate(streams):
                    rhs0 = rhs_ap(X, st, d, cy - y0, 1, W8, 1)
                    rhs1 = rhs_ap(X, st, d, cy + 1 - y0, 1, W8, 1)
                    w_ = wt[(8, 0, 0, 0, si)][:]
                    last = si == len(streams) - 1
                    nc.tensor.matmul(ps[0:64, 0:W8], w_, rhs0, start=(si == 0),
                                     stop=last, tile_position=(0, 0),
                                     skip_group_check=True)
                    nc.tensor.matmul(ps[64:128, 0:W8], w_, rhs1, start=(si == 0),
                                     stop=last, tile_position=(0, 64),
                                     skip_group_check=True)
                pi = (cy - st0) // 2
                act_sign(8, 0, ps[:, 0:W8],
                         t[:, pi:pi + 1, 1:1 + W8].rearrange(
                             "p a w -> p (a w)"), 128)
            nc.sync.dma_start(
                bufs[8].ap()[:, 1 + st0:1 + st0 + 2 * (SRp - 1) + 1:2, :],
                t[0:64, :, :])
            nc.sync.dma_start(
                bufs[8].ap()[:, 2 + st0:2 + st0 + 2 * (SRp - 1) + 1:2, :],
                t[64:128, :, :])


def _emit_tconv(nc, tc, geom, l, bufs, wt, psum, inp, stg, rhs_ap, act_sign):
    d = geom[l]
    cin, cout, s_, tr = CFG[l]
    nhalf = cout // 128
    nslab = cin // 128
    Wq, nq, SRq = d['Wq'], d['nq'], d['SRq']
    Bq = d['B'] // 2
    for Y0 in range(0, d['Hout'], d['B']):
        u0 = Y0 // 2
        Rt = Bq + 1
        Xs = []
        for slab in range(nslab):
            X = inp.tile([128, Rt, d['Wpin']], FP8, tag=f"in{l}_{slab}")
            nc.sync.dma_start(
                X[:], bufs[l - 1].ap()[128 * slab:128 * slab + 128,
                                       u0 + 1:u0 + 1 + Rt, :])
            Xs.append(X)
        for q0 in range(u0, u0 + Bq, SRq):
            stgs = []
            for half in range(nhalf):
                t = stg.tile([128, 2 * SRq, d['Wpout']], FP8, tag=f"stg{l}_{half}")
                nc.gpsimd.memset(t[:, :, 0:1], 0.0)
                nc.gpsimd.memset(t[:, :, d['Wout'] + 1:], 0.0)
                stgs.append(t)
            for ci, (py, px, streams) in enumerate(TCONV_CLASSES):
                for qy in range(q0, q0 + SRq, nq):
                    for half in range(nhalf):
                        ps = psum.tile([128, 512], F32, tag="ps")
                        work = []
                        for slab in range(nslab):
                            for si, taps in enumerate(streams):
                                work.append((slab, si, taps))
                        for i, (slab, si, taps) in enumerate(work):
                            a, b_, da, db = taps[0]
                            st = dict(kind='t', K=128, p0=0, row_off=da,
                                      col_off=db + 1,
                                      jrows=(1 if len(taps) == 2 else None))
                            rhs = rhs_ap(Xs[slab], st, d, qy - u0, nq, Wq, 1)
                            nc.tensor.matmul(
                                ps[0:128, 0:nq * Wq],
                                wt[(l, half, slab, ci, si)][:], rhs,
                                start=(i == 0), stop=(i == len(work) - 1),
                                perf_mode=(DR if len(taps) == 2 else None))
                        r0 = py + 2 * (qy - q0)
                        so = stgs[half][
                            0:128,
                            r0: r0 + 2 * (nq - 1) + 1: 2,
                            1 + px: 1 + px + 2 * (Wq - 1) + 1: 2]
                        act_sign(l, half,
                                 ps[0:128, 0:nq * Wq].rearrange(
                                     "m (r w) -> m r w", r=nq), so, 128)
            for half in range(nhalf):
                nc.sync.dma_start(
                    bufs[l].ap()[128 * half:128 * half + 128,
                                 1 + 2 * q0: 1 + 2 * q0 + 2 * SRq, :],
                    stgs[half][:, :, :])


# ---------------------------------------------------------------------------
# entry point
# ---------------------------------------------------------------------------

_CACHE = {}


def _prepare(inputs, dbg=False):
    H0 = inputs['x'].shape[2]
    geom = make_geom(H0)
    a0, th, s9, t9 = _host_l0_and_thresholds(inputs)
    wpack, woffsets = _weight_arrays(inputs)
    bpack, colidx = _bias_pack(th, s9, t9)
    a0p = _pack_a0(a0)
    key = (H0, dbg)
    if key not in _CACHE:
        _CACHE[key] = build_nc(geom, woffsets, colidx, bpack.shape[1],
                               wpack.size, dbg=dbg)
    nc = _CACHE[key]
    in_maps = [{"a0": a0p[c], "wpack": wpack, "bpack": bpack}
               for c in range(NCORES)]
    return nc, in_maps, geom


def run(inputs, dbg=False, trace=False):
    nc, in_maps, geom = _prepare(inputs, dbg=dbg)
    res = run_bass_kernel_spmd(nc, in_maps, core_ids=list(range(NCORES)),
                               trace=trace)
    Hlog = geom['sizes'][9]
    nb = geom[9]['Hout'] // geom[9]['B']
    sub = geom[9]['B'] // 8
    full = np.empty((8, 19, Hlog, Hlog), np.float32)
    fv = full.reshape(8, 19, nb, sub, 8, Hlog)
    for j in range(NCORES):
        o = res.results[j]["out"].reshape(nb, 8, 19, sub, Hlog)
        fv[:, :, :, :, j, :] = o.transpose(1, 2, 0, 3, 4)
    return full, res


def kernel(**inputs) -> np.ndarray:
    full, _ = run(inputs)
    return full


# revision 12
# speedup vs baseline: 1.0120x; 1.0120x over previous
"""Trainium2 distributed kernel for nn_BDEN_16621523435784 (binarized dense CNN).

Strategy (data parallel, 1 sample per NeuronCore, 8 cores):
- Layer 0 (3-channel f32 input conv) is computed on host with the exact same
  eager-jax ops as the reference (bit-exact; ~0.1% of total FLOPs). The network
  is chaotic: one flipped sign anywhere changes the final output completely, so
  everything must be bit-exact.
- Layers 1..8 on device: activations/weights are +-1 (fp8, exact), conv sums are
  exact integers (fp32 PSUM accumulate). BN+sign collapses to an integer
  threshold compare per channel, fused into one ScalarE Sign activation
  (bias = +-(0.5 - T_c), scale = +-1), thresholds precomputed on host with
  bit-exact jax arithmetic.
- Layer 9 produces f32 logits (affine on integer sums), written into per-band
  AllToAll buffers (output row y belongs to core y mod 8), 8 pipelined
  AllToAlls + on-device softmax over the batch axis.
"""

import os

os.environ.setdefault("JAX_PLATFORMS", "")

import numpy as np
import ml_dtypes

import concourse.bass as bass
import concourse.tile as tile
from concourse import bacc, mybir
from concourse.bass_utils import run_bass_kernel_spmd

FP8 = mybir.dt.float8e4
F32 = mybir.dt.float32
NP8 = ml_dtypes.float8_e4m3
AF = mybir.ActivationFunctionType
DR = mybir.MatmulPerfMode.DoubleRow

NCORES = 8
EPS = 1e-5

# (cin, cout, stride, transposed) for the 10 binarized conv layers
CFG = [(3, 64, 2, False), (64, 64, 1, False), (64, 128, 2, False), (128, 128, 1, False),
       (128, 256, 1, False), (256, 256, 2, True), (256, 128, 1, False), (128, 128, 2, True),
       (128, 64, 1, False), (64, 19, 1, False)]

# tconv output-parity classes: (py, px, [taps (a, b, da, db)], doublerow?)
# out(2u+py, 2v+px) = sum_taps wf[:, :, a, b] @ in[u+da, v+db]
TCONV_CLASSES = [
    (0, 0, [[(1, 1, 0, 0)]]),
    (0, 1, [[(1, 0, 0, 0)], [(1, 2, 0, 1)]]),
    (1, 0, [[(0, 1, 0, 0), (2, 1, 1, 0)]]),
    (1, 1, [[(0, 0, 0, 0), (2, 0, 1, 0)], [(0, 2, 0, 1), (2, 2, 1, 1)]]),
]


def pad16(x):
    return (x + 15) // 16 * 16


def make_geom(H0):
    g = {}
    H = H0 // 2
    sizes = [H]
    for l in range(1, 10):
        cin, cout, s, tr = CFG[l]
        H = H * 2 if tr else H // s
        sizes.append(H)
    for l in range(1, 10):
        cin, cout, s, tr = CFG[l]
        Hin, Hout = sizes[l - 1], sizes[l]
        d = dict(cin=cin, cout=cout, s=s, tr=tr, Hin=Hin, Win=Hin, Hout=Hout,
                 Wout=Hout, Wpin=pad16(Hin + 2), Wpout=pad16(Hout + 2),
                 dup=(cin == 64))
        if tr:
            d['Wq'] = Hout // 2
            d['nq'] = max(1, min(512 // d['Wq'], Hout // 2))
        else:
            d['nr'] = max(1, min(512 // Hout, Hout))
        if l == 5:
            B = Hout
        elif l in (1, 7):
            B = min(128, Hout)
        elif l in (2, 6, 8, 9):
            B = min(64, Hout)
        else:
            B = Hout
        d['B'] = B
        if tr:
            d['SRq'] = min(8, B // 2)
            d['nq'] = min(d['nq'], d['SRq'])
        elif l == 8:
            d['SRp'] = min(8, B // 2)
        elif l != 9:
            d['SR'] = min(max(d['nr'] * 4, 8), B)
        g[l] = d
    g['sizes'] = sizes
    g['H0'] = H0
    return g


# ---------------------------------------------------------------------------
# Stream plans. A stream: (wkey_suffix si, kind, K, row_off, col_off, jrows)
#   kind 'dr'/'dupdr' use DoubleRow; row_off = tile-row offset (ky/da term),
#   col_off = buffer-col offset (kx/db term), jrows = j-dim step in input rows.
# ---------------------------------------------------------------------------

def conv_streams(l):
    """L3/L4/L6 per-slab streams: 3 DoubleRow (ky 0+1) + 3 plain (ky=2)."""
    out = []
    for kx in range(3):
        out.append(dict(kind='dr', K=128, p0=0, row_off=0, col_off=kx, jrows=1))
    for kx in range(3):
        out.append(dict(kind='plain', K=128, p0=0, row_off=2, col_off=kx, jrows=None,
                        ky=2, kx=kx))
    return out


def dupdr_streams():
    return [dict(kind='dupdr', K=128, p0=0, row_off=0, col_off=kx, jrows=2)
            for kx in range(3)]


def l8_streams():
    return [dict(kind='plain', K=128, p0=0, row_off=ky, col_off=kx, jrows=None,
                 ky=ky, kx=kx) for ky in range(3) for kx in range(3)]


def l9_streams():
    out = []
    for kx in range(3):
        out.append(dict(kind='dupk128', K=128, p0=0, row_off=0, col_off=kx, jrows=None))
    for kx in range(3):
        out.append(dict(kind='dupk64', K=64, p0=0, row_off=2, col_off=kx, jrows=None))
    return out


# ---------------------------------------------------------------------------
# Host-side preparation
# ---------------------------------------------------------------------------

def _host_l0_and_thresholds(inputs):
    import jax
    import jax.numpy as jnp
    from jax import lax

    cpu = jax.devices("cpu")[0]
    with jax.default_device(cpu):
        return _host_l0_impl(inputs, jnp, lax)


def _host_l0_impl(inputs, jnp, lax):

    DN = ('NCHW', 'OIHW', 'NCHW')
    x = jnp.asarray(inputs['x'])
    w0 = jnp.asarray(inputs['w0'])
    bnp0 = jnp.asarray(inputs['bnp0'])

    wb0 = jnp.sign(w0)
    h = lax.conv_general_dilated(x, wb0, (2, 2), ((1, 1), (1, 1)),
                                 dimension_numbers=DN)
    g, b, m, v = bnp0[0], bnp0[1], bnp0[2], bnp0[3]
    h = g[:, None, None] * (h - m[:, None, None]) * lax.rsqrt(v[:, None, None] + EPS) \
        + b[:, None, None]
    a0 = np.asarray(jnp.sign(h), dtype=np.float32)

    th = {}
    for l in range(1, 9):
        cin, cout, s, tr = CFG[l]
        Kmax = cin * 9
        bnp = jnp.asarray(inputs[f'bnp{l}'])
        ks = jnp.arange(-Kmax, Kmax + 1, dtype=jnp.float32)
        xg = jnp.broadcast_to(ks[None, :, None], (cout, 2 * Kmax + 1, 1))
        g, b, m, v = bnp[0], bnp[1], bnp[2], bnp[3]
        vals = g[:, None, None] * (xg - m[:, None, None]) \
            * lax.rsqrt(v[:, None, None] + EPS) + b[:, None, None]
        sg = np.asarray(jnp.sign(vals))[:, :, 0]
        assert not (sg == 0).any(), f"layer {l}: bn output exactly 0 on integer grid"
        pos = sg > 0
        scale = np.empty(cout, np.float32)
        bias = np.empty(cout, np.float32)
        for c in range(cout):
            p = pos[c]
            if p.all():
                scale[c], bias[c] = 1.0, float(Kmax + 1.5)
            elif not p.any():
                scale[c], bias[c] = 1.0, float(-(Kmax + 1.5))
            else:
                dd = np.diff(p.astype(np.int8))
                nz = np.nonzero(dd)[0]
                assert len(nz) == 1, f"layer {l} ch {c}: non-monotone bn sign"
                idx = int(nz[0])
                if dd[idx] == 1:   # sign=+1 iff k >= T
                    T = float(idx + 1 - Kmax)
                    scale[c], bias[c] = 1.0, 0.5 - T
                else:              # sign=+1 iff k <= T
                    T = float(idx - Kmax)
                    scale[c], bias[c] = -1.0, T + 0.5
        th[l] = (scale, bias)

    bnp9 = np.asarray(inputs['bnp9'], np.float64)
    g, b, m, v = bnp9[0], bnp9[1], bnp9[2], bnp9[3]
    s9 = (g / np.sqrt(v + EPS)).astype(np.float32)
    t9 = (b - m * g / np.sqrt(v + EPS)).astype(np.float32)
    return a0, th, s9, t9


def _weight_arrays(inputs):
    """All stream lhsT arrays (fp8) flat-packed; offsets[(l,half,slab,ci,si)]."""
    blocks, offsets, cursor = [], {}, [0]

    def emit(key, arr):
        a = np.ascontiguousarray(arr.astype(NP8))
        offsets[key] = (cursor[0], a.shape)
        blocks.append(a.reshape(-1))
        cursor[0] += a.size

    for l in range(1, 10):
        cin, cout, s, tr = CFG[l]
        w = np.sign(np.asarray(inputs[f'w{l}'], np.float32))
        if tr:
            wf = np.flip(w, (2, 3)).transpose(1, 0, 2, 3)  # [cout, cin, 3, 3]
            for half in range(cout // 128):
                M = 128
                hs = slice(128 * half, 128 * half + M)
                for slab in range(cin // 128):
                    ss = slice(128 * slab, 128 * slab + 128)
                    for ci, (py, px, streams) in enumerate(TCONV_CLASSES):
                        for si, taps in enumerate(streams):
                            if len(taps) == 1:
                                a, b_, da, db = taps[0]
                                emit((l, half, slab, ci, si), wf[hs, ss, a, b_].T)
                            else:
                                arr = np.zeros((128, 2, M), np.float32)
                                for j, (a, b_, da, db) in enumerate(taps):
                                    arr[:, j, :] = wf[hs, ss, a, b_].T
                                emit((l, half, slab, ci, si), arr)
        elif l in (1, 2):
            for half in range((cout + 127) // 128):
                M = min(128, cout - 128 * half)
                hs = slice(128 * half, 128 * half + M)
                for si, st in enumerate(dupdr_streams()):
                    kx = st['col_off']
                    arr = np.zeros((128, 2, M), np.float32)
                    arr[:64, 0, :] = w[hs, :, 0, kx].T
                    arr[:64, 1, :] = w[hs, :, 2, kx].T
                    arr[64:, 0, :] = w[hs, :, 1, kx].T
                    emit((l, half, 0, 0, si), arr)
        elif l == 8:
            for si, st in enumerate(l8_streams()):
                emit((8, 0, 0, 0, si), w[:, :, st['ky'], st['kx']].T)
        elif l == 9:
            for si, st in enumerate(l9_streams()):
                kx = st['col_off']
                if st['kind'] == 'dupk128':
                    arr = np.zeros((128, 19), np.float32)
                    arr[:64] = w[:, :, 0, kx].T
                    arr[64:] = w[:, :, 1, kx].T
                else:
                    arr = w[:, :, 2, kx].T
                emit((9, 0, 0, 0, si), arr)
        else:  # L3, L4, L6
            for half in range((cout + 127) // 128):
                M = min(128, cout - 128 * half)
                hs = slice(128 * half, 128 * half + M)
                for slab in range(max(1, cin // 128)):
                    ss = slice(128 * slab, 128 * slab + 128)
                    for si, st in enumerate(conv_streams(l)):
                        if st['kind'] == 'dr':
                            kx = st['col_off']
                            arr = np.zeros((128, 2, M), np.float32)
                            arr[:, 0, :] = w[hs, ss, 0, kx].T
                            arr[:, 1, :] = w[hs, ss, 1, kx].T
                        else:
                            arr = w[hs, ss, st['ky'], st['kx']].T
                        emit((l, half, slab, 0, si), arr)
    wpack = np.concatenate(blocks)
    if wpack.size % 16:
        wpack = np.concatenate([wpack, np.zeros(16 - wpack.size % 16, NP8)])
    return wpack, offsets


def _bias_pack(th, s9, t9):
    cols, colidx = [], {}

    def add(name, vec):
        colidx[name] = len(cols)
        v = np.zeros(128, np.float32)
        v[:len(vec)] = vec
        cols.append(v)

    for l in range(1, 9):
        scale, bias = th[l]
        cout = CFG[l][1]
        if l == 8:
            # col-tiling x2: row pair shares the bank, channels replicated at +64
            add(('scale', 8, 0), np.concatenate([scale, scale]))
            add(('bias', 8, 0), np.concatenate([bias, bias]))
            continue
        for half in range((cout + 127) // 128):
            sl = slice(128 * half, min(cout, 128 * half + 128))
            add(('scale', l, half), scale[sl])
            add(('bias', l, half), bias[sl])
    s9r, t9r = np.zeros(128, np.float32), np.zeros(128, np.float32)
    for c in range(4):
        s9r[32 * c:32 * c + 19] = s9
        t9r[32 * c:32 * c + 19] = t9
    add(('scale', 9, 0), s9r)
    add(('bias', 9, 0), t9r)
    return np.stack(cols, axis=1), colidx


def _pack_a0(a0):
    H = a0.shape[2]
    Wp = pad16(H + 2)
    out = np.zeros((8, 64, H + 4, Wp), NP8)
    out[:, :, 1:H + 1, 1:H + 1] = a0.astype(NP8)
    return out


def with_jdim(ap, step_elems):
    """Insert dim 1 with [step_elems, 2] into a 3D ap [K, rows, cols]."""
    shp = ap.shape
    a = ap.unsqueeze(1).broadcast_to((shp[0], 2) + tuple(shp[1:]))
    a.ap[1] = [step_elems, 2]
    return a


# ---------------------------------------------------------------------------
# Device graph
# ---------------------------------------------------------------------------

def build_nc(geom, woffsets, colidx, nbias, nwpack, dbg=False):
    Hlog = geom['sizes'][9]
    nc = bacc.Bacc("TRN2", target_bir_lowering=False, debug=False)

    a0_p = nc.declare_dram_parameter(
        "a0", [64, geom[1]['Hin'] + 4, geom[1]['Wpin']], FP8, isOutput=False)
    wpack_p = nc.declare_dram_parameter("wpack", [nwpack], FP8, isOutput=False)
    bpack_p = nc.declare_dram_parameter("bpack", [128, nbias], F32, isOutput=False)
    nb9_ = geom[9]['Hout'] // geom[9]['B']
    sub_ = geom[9]['B'] // 8
    out_p = nc.declare_dram_parameter("out", [nb9_, 8, 19, sub_, Hlog], F32,
                                      isOutput=True)

    bufs = {0: a0_p}
    for l in range(1, 9):
        d = geom[l]
        bufs[l] = nc.dram_tensor(f"act{l}", [d['cout'], d['Hout'] + 4, d['Wpout']], FP8)
    nb9 = geom[9]['Hout'] // geom[9]['B']
    sub = geom[9]['B'] // 8
    a2a_in = [nc.dram_tensor(f"a2ain{b}", [8, 19, sub, Hlog], F32) for b in range(nb9)]
    a2a_out = [nc.dram_tensor(f"a2aout{b}", [8, 19, sub, Hlog], F32) for b in range(nb9)]
    dbg_p = {}
    if dbg:
        for l in range(1, 9):
            d = geom[l]
            dbg_p[l] = nc.declare_dram_parameter(
                f"dbg{l}", [d['cout'], d['Hout'] + 4, d['Wpout']], FP8, isOutput=True)
        dbg_p[9] = nc.declare_dram_parameter(
            "dbg9", [nb9, 8, 19, sub, Hlog], F32, isOutput=True)

    with tile.TileContext(nc) as tc:
        with (
            tc.tile_pool(name="weights", bufs=1) as wpool,
            tc.tile_pool(name="psum", bufs=6, space=bass.MemorySpace.PSUM) as psum,
        ):
            # --- weights + bias ---
            wt = {}
            for key, (off, shp) in woffsets.items():
                t = wpool.tile(list(shp), FP8, tag=f"w{key}")
                flat = int(np.prod(shp))
                src = wpack_p.ap()[off:off + flat].rearrange("(k r) -> k r", k=shp[0])
                dst = t[:] if len(shp) == 2 else t[:].rearrange("k j m -> k (j m)")
                nc.sync.dma_start(dst, src)
                wt[key] = t
            bp = wpool.tile([128, nbias], F32, tag="bpack")
            nc.sync.dma_start(bp[:], bpack_p[:])

            # --- zero halo rows of act buffers ---
            zt = wpool.tile([128, 3 * 544], FP8, tag="zeros")
            nc.gpsimd.memset(zt[:], 0.0)
            for l in range(1, 9):
                d = geom[l]
                for slab in range(0, d['cout'], 128):
                    C = min(128, d['cout'] - slab)
                    bb = bufs[l].ap()[slab:slab + C]
                    Wp, H = d['Wpout'], d['Hout']
                    nc.sync.dma_start(
                        bb[:, 0:1, :].rearrange("c a b -> c (a b)"), zt[0:C, 0:Wp])
                    nc.sync.dma_start(
                        bb[:, H + 1:H + 4, :].rearrange("c a b -> c (a b)"),
                        zt[0:C, 0:3 * Wp])

            def rhs_ap(X, st, d, local_row, nr, W, stride):
                r0 = local_row + st['row_off']
                c0 = st['col_off']
                base = X[st['p0']:st['p0'] + st['K'],
                         r0: r0 + stride * (nr - 1) + 1: stride,
                         c0: c0 + stride * (W - 1) + 1: stride]
                if st['jrows'] is not None:
                    return with_jdim(base, st['jrows'] * d['Wpin'])
                return base

            def act_sign(l, half, ps_ap, out_ap, M):
                ic = colidx[('scale', l, half)]
                jc = colidx[('bias', l, half)]
                nc.scalar.activation(out_ap, ps_ap, AF.Sign,
                                     bias=bp[0:M, jc:jc + 1], scale=bp[0:M, ic:ic + 1])

            # ----------------------------------------------------------------
            # layers 1..8
            for l in range(1, 9):
                d = geom[l]
                cin, cout, s, tr = CFG[l]
                with (
                    tc.tile_pool(name=f"in{l}", bufs=2) as inp,
                    tc.tile_pool(name=f"stg{l}", bufs=3) as stg,
                ):
                    if tr:
                        _emit_tconv(nc, tc, geom, l, bufs, wt, psum, inp, stg,
                                    rhs_ap, act_sign)
                    elif l == 8:
                        _emit_l8(nc, geom, bufs, wt, psum, inp, stg, rhs_ap, act_sign)
                    else:
                        _emit_conv(nc, geom, l, bufs, wt, psum, inp, stg,
                                   rhs_ap, act_sign)

            # ----------------------------------------------------------------
            # layer 9 + pipelined A2A + softmax
            d = geom[9]
            with (
                tc.tile_pool(name="in9", bufs=2) as inp,
                tc.tile_pool(name="stg9", bufs=4) as stg,
                tc.tile_pool(name="smx", bufs=2) as smx,
            ):
                B9 = d['B']
                ic9 = colidx[('scale', 9, 0)]
                jc9 = colidx[('bias', 9, 0)]
                for bi, y0 in enumerate(range(0, d['Hout'], B9)):
                    Rt = B9 + 2
                    X = inp.tile([128, Rt, d['Wpin']], FP8, tag="in9")
                    nc.sync.dma_start(X[0:64, :, :], bufs[8].ap()[:, y0:y0 + Rt, :])
                    nc.sync.dma_start(X[64:128, :, :],
                                      bufs[8].ap()[:, y0 + 1:y0 + 1 + Rt, :])
                    W9 = d['Wout']
                    for cy in range(y0, y0 + B9, 4):
                        ps = psum.tile([128, 512], F32, tag="ps")
                        streams = l9_streams()
                        for si, st in enumerate(streams):
                            for c in range(4):
                                rhs = rhs_ap(X, st, d, cy + c - y0, 1, W9, 1)
                                nc.tensor.matmul(
                                    ps[32 * c:32 * c + 19, 0:W9],
                                    wt[(9, 0, 0, 0, si)][:], rhs,
                                    start=(si == 0), stop=(si == len(streams) - 1),
                                    tile_position=(0, 32 * c),
                                    skip_group_check=True)
                        st_t = stg.tile([128, 512], F32, tag="stg9")
                        nc.scalar.activation(st_t[:, 0:W9], ps[:, 0:W9], AF.Identity,
                                             bias=bp[:, jc9:jc9 + 1],
                                             scale=bp[:, ic9:ic9 + 1])
                        for c in range(4):
                            y = cy + c
                            j = y % 8
                            r = (y - y0 - j) // 8
                            dst = a2a_in[bi].ap()[j:j + 1, :, r:r + 1, :].squeeze()
                            nc.scalar.dma_start(dst, st_t[32 * c:32 * c + 19, 0:W9])
                    nc.gpsimd.collective_compute(
                        "AllToAll", mybir.AluOpType.bypass,
                        replica_groups=[list(range(NCORES))],
                        ins=[a2a_in[bi].ap().opt()],
                        outs=[a2a_out[bi].ap().opt()],
                    )
                    # softmax over samples for this band
                    F = 19 * sub * Hlog
                    assert F % 128 == 0
                    FD = F // 128
                    Ts = []
                    for sx in range(8):
                        t = smx.tile([128, FD], F32, tag=f"sm_in{sx}")
                        nc.gpsimd.dma_start(
                            t[:], a2a_out[bi].ap()[sx:sx + 1].rearrange(
                                "s a b c -> (s a b c)").rearrange("(p f) -> p f", p=128))
                        Ts.append(t)
                    mx = smx.tile([128, FD], F32, tag="sm_mx")
                    nc.vector.tensor_max(mx[:], Ts[0][:], Ts[1][:])
                    for sx in range(2, 8):
                        nc.vector.tensor_max(mx[:], mx[:], Ts[sx][:])
                    Es = []
                    for sx in range(8):
                        e = smx.tile([128, FD], F32, tag=f"sm_e{sx}")
                        nc.vector.tensor_sub(e[:], Ts[sx][:], mx[:])
                        nc.scalar.activation(e[:], e[:], AF.Exp)
                        Es.append(e)
                    S = smx.tile([128, FD], F32, tag="sm_s")
                    nc.vector.tensor_add(S[:], Es[0][:], Es[1][:])
                    for sx in range(2, 8):
                        nc.vector.tensor_add(S[:], S[:], Es[sx][:])
                    R = smx.tile([128, FD], F32, tag="sm_r")
                    nc.vector.reciprocal(R[:], S[:])
                    for sx in range(8):
                        nc.vector.tensor_mul(Es[sx][:], Es[sx][:], R[:])
                        nc.gpsimd.dma_start(
                            out_p.ap()[bi:bi + 1, sx:sx + 1].rearrange(
                                "x s a b c -> (x s a b c)").rearrange(
                                "(p f) -> p f", p=128),
                            Es[sx][:])

            if dbg:
                for l in range(1, 9):
                    nc.sync.dma_start(dbg_p[l].ap(), bufs[l].ap())
                for b in range(nb9):
                    nc.sync.dma_start(dbg_p[9].ap()[b:b + 1].squeeze(), a2a_in[b].ap())

    nc.compile()
    return nc


def _emit_conv(nc, geom, l, bufs, wt, psum, inp, stg, rhs_ap, act_sign):
    """L1, L2 (dup+DoubleRow), L3, L4, L6 (DoubleRow pairs + plain)."""
    d = geom[l]
    cin, cout, s, tr = CFG[l]
    nhalf = (cout + 127) // 128
    nslab = max(1, cin // 128)
    nr, SR = d['nr'], d['SR']
    for y0 in range(0, d['Hout'], d['B']):
        br0 = s * y0
        Rt = s * (d['B'] - 1) + 3
        Xs = []
        if d['dup']:
            X = inp.tile([128, Rt, d['Wpin']], FP8, tag=f"in{l}")
            nc.sync.dma_start(X[0:64, :, :], bufs[l - 1].ap()[:, br0:br0 + Rt, :])
            nc.sync.dma_start(X[64:128, :, :],
                              bufs[l - 1].ap()[:, br0 + 1:br0 + 1 + Rt, :])
            Xs = [X]
        else:
            for slab in range(nslab):
                X = inp.tile([128, Rt, d['Wpin']], FP8, tag=f"in{l}_{slab}")
                nc.sync.dma_start(
                    X[:], bufs[l - 1].ap()[128 * slab:128 * slab + 128,
                                           br0:br0 + Rt, :])
                Xs.append(X)
        for st0 in range(y0, y0 + d['B'], SR):
            stgs = []
            for half in range(nhalf):
                M = min(128, cout - 128 * half)
                t = stg.tile([128, SR, d['Wpout']], FP8, tag=f"stg{l}_{half}")
                nc.gpsimd.memset(t[0:M, :, 0:1], 0.0)
                nc.gpsimd.memset(t[0:M, :, d['Wout'] + 1:], 0.0)
                stgs.append(t)
            for cy in range(st0, st0 + SR, nr):
                for half in range(nhalf):
                    M = min(128, cout - 128 * half)
                    ps = psum.tile([128, 512], F32, tag="ps")
                    work = []
                    for slab in range(nslab):
                        sts = dupdr_streams() if d['dup'] else conv_streams(l)
                        for si, st in enumerate(sts):
                            work.append((slab, si, st))
                    for i, (slab, si, st) in enumerate(work):
                        rhs = rhs_ap(Xs[slab], st, d, s * (cy - y0), nr, d['Wout'], s)
                        nc.tensor.matmul(
                            ps[0:M, 0:nr * d['Wout']],
                            wt[(l, half, slab, 0, si)][:], rhs,
                            start=(i == 0), stop=(i == len(work) - 1),
                            perf_mode=(DR if st['jrows'] is not None else None))
                    act_sign(l, half,
                             ps[0:M, 0:nr * d['Wout']].rearrange(
                                 "m (r w) -> m r w", r=nr),
                             stgs[half][0:M, cy - st0:cy - st0 + nr, 1:1 + d['Wout']],
                             M)
            for half in range(nhalf):
                M = min(128, cout - 128 * half)
                nc.sync.dma_start(
                    bufs[l].ap()[128 * half:128 * half + M, 1 + st0:1 + st0 + SR, :],
                    stgs[half][0:M, :, :])


def _emit_l8(nc, geom, bufs, wt, psum, inp, stg, rhs_ap, act_sign):
    """L8: 128->64, col-tiling x2 (two output rows per psum bank)."""
    d = geom[8]
    SRp = d['SRp']
    streams = l8_streams()
    for y0 in range(0, d['Hout'], d['B']):
        Rt = d['B'] + 2
        X = inp.tile([128, Rt, d['Wpin']], FP8, tag="in8")
        nc.sync.dma_start(X[:], bufs[7].ap()[:, y0:y0 + Rt, :])
        for st0 in range(y0, y0 + d['B'], 2 * SRp):
            t = stg.tile([128, SRp, d['Wpout']], FP8, tag="stg8")
            nc.gpsimd.memset(t[:, :, 0:1], 0.0)
            nc.gpsimd.memset(t[:, :, d['Wout'] + 1:], 0.0)
            for cy in range(st0, st0 + 2 * SRp, 2):
                ps = psum.tile([128, 512], F32, tag="ps")
                W8 = d['Wout']
                for si, st in enumerate(streams):
                    rhs0 = rhs_ap(X, st, d, cy - y0, 1, W8, 1)
                    rhs1 = rhs_ap(X, st, d, cy + 1 - y0, 1, W8, 1)
                    w_ = wt[(8, 0, 0, 0, si)][:]
                    last = si == len(streams) - 1
                    nc.tensor.matmul(ps[0:64, 0:W8], w_, rhs0, start=(si == 0),
                                     stop=last, tile_position=(0, 0),
                                     skip_group_check=True)
                    nc.tensor.matmul(ps[64:128, 0:W8], w_, rhs1, start=(si == 0),
                                     stop=last, tile_position=(0, 64),
                                     skip_group_check=True)
                pi = (cy - st0) // 2
                act_sign(8, 0, ps[:, 0:W8],
                         t[:, pi:pi + 1, 1:1 + W8].rearrange(
                             "p a w -> p (a w)"), 128)
            nc.sync.dma_start(
                bufs[8].ap()[:, 1 + st0:1 + st0 + 2 * (SRp - 1) + 1:2, :],
                t[0:64, :, :])
            nc.sync.dma_start(
                bufs[8].ap()[:, 2 + st0:2 + st0 + 2 * (SRp - 1) + 1:2, :],
                t[64:128, :, :])


def _emit_tconv(nc, tc, geom, l, bufs, wt, psum, inp, stg, rhs_ap, act_sign):
    d = geom[l]
    cin, cout, s_, tr = CFG[l]
    nhalf = cout // 128
    nslab = cin // 128
    Wq, nq, SRq = d['Wq'], d['nq'], d['SRq']
    Bq = d['B'] // 2
    for Y0 in range(0, d['Hout'], d['B']):
        u0 = Y0 // 2
        Rt = Bq + 1
        Xs = []
        for slab in range(nslab):
            X = inp.tile([128, Rt, d['Wpin']], FP8, tag=f"in{l}_{slab}")
            nc.sync.dma_start(
                X[:], bufs[l - 1].ap()[128 * slab:128 * slab + 128,
                                       u0 + 1:u0 + 1 + Rt, :])
            Xs.append(X)
        for q0 in range(u0, u0 + Bq, SRq):
            stgs = []
            for half in range(nhalf):
                t = stg.tile([128, 2 * SRq, d['Wpout']], FP8, tag=f"stg{l}_{half}")
                nc.gpsimd.memset(t[:, :, 0:1], 0.0)
                nc.gpsimd.memset(t[:, :, d['Wout'] + 1:], 0.0)
                stgs.append(t)
            for ci, (py, px, streams) in enumerate(TCONV_CLASSES):
                for qy in range(q0, q0 + SRq, nq):
                    for half in range(nhalf):
                        ps = psum.tile([128, 512], F32, tag="ps")
                        work = []
                        for slab in range(nslab):
                            for si, taps in enumerate(streams):
                                work.append((slab, si, taps))
                        for i, (slab, si, taps) in enumerate(work):
                            a, b_, da, db = taps[0]
                            st = dict(kind='t', K=128, p0=0, row_off=da,
                                      col_off=db + 1,
                                      jrows=(1 if len(taps) == 2 else None))
                            rhs = rhs_ap(Xs[slab], st, d, qy - u0, nq, Wq, 1)
                            nc.tensor.matmul(
                                ps[0:128, 0:nq * Wq],
                                wt[(l, half, slab, ci, si)][:], rhs,
                                start=(i == 0), stop=(i == len(work) - 1),
                                perf_mode=(DR if len(taps) == 2 else None))
                        r0 = py + 2 * (qy - q0)
                        so = stgs[half][
                            0:128,
                            r0: r0 + 2 * (nq - 1) + 1: 2,
                            1 + px: 1 + px + 2 * (Wq - 1) + 1: 2]
                        act_sign(l, half,
                                 ps[0:128, 0:nq * Wq].rearrange(
                                     "m (r w) -> m r w", r=nq), so, 128)
            for half in range(nhalf):
                nc.sync.dma_start(
                    bufs[l].ap()[128 * half:128 * half + 128,
                                 1 + 2 * q0: 1 + 2 * q0 + 2 * SRq, :],
                    stgs[half][:, :, :])


# ---------------------------------------------------------------------------
# entry point
# ---------------------------------------------------------------------------

_CACHE = {}


def _prepare(inputs, dbg=False):
    H0 = inputs['x'].shape[2]
    geom = make_geom(H0)
    a0, th, s9, t9 = _host_l0_and_thresholds(inputs)
    wpack, woffsets = _weight_arrays(inputs)
    bpack, colidx = _bias_pack(th, s9, t9)
    a0p = _pack_a0(a0)
    key = (H0, dbg)
    if key not in _CACHE:
        _CACHE[key] = build_nc(geom, woffsets, colidx, bpack.shape[1],
                               wpack.size, dbg=dbg)
    nc = _CACHE[key]
    in_maps = [{"a0": a0p[c], "wpack": wpack, "bpack": bpack}
               for c in range(NCORES)]
    return nc, in_maps, geom


def run(inputs, dbg=False, trace=False):
    nc, in_maps, geom = _prepare(inputs, dbg=dbg)
    res = run_bass_kernel_spmd(nc, in_maps, core_ids=list(range(NCORES)),
                               trace=trace)
    Hlog = geom['sizes'][9]
    nb = geom[9]['Hout'] // geom[9]['B']
    sub = geom[9]['B'] // 8
    full = np.empty((8, 19, Hlog, Hlog), np.float32)
    fv = full.reshape(8, 19, nb, sub, 8, Hlog)
    for j in range(NCORES):
        o = res.results[j]["out"].reshape(nb, 8, 19, sub, Hlog)
        fv[:, :, :, :, j, :] = o.transpose(1, 2, 0, 3, 4)
    return full, res


def kernel(**inputs) -> np.ndarray:
    full, _ = run(inputs)
    return full
